# revision 1
# baseline (speedup 1.0000x reference)
"""Trainium2 Bass kernel for the per-gene sparse-decoder MLP.

Math (per gene g): h = selu(features[:, tf_idx[g]] @ W1[g].T); h = selu(h @ Wm[i,g].T) x2;
out[:, g] = h @ Wf[g].  Genes are independent -> shard G=20000 over 8 cores (2500 each).

Device mapping per core (gene dim padded 2500 -> 2560):
  - Activations live as [node-rows on partitions, batch on free].  Per "unit" of
    32 genes: L1 is 4 matmuls of [128c=(8 genes x 16k), M=64=(8g x 8w), N=256]
    with block-diagonal lhsT; L2/L3/Lf are per-16-gene-group matmul PAIRS sharing
    one block-diagonal lhsT, accumulating two SELU streams into PSUM:
        selu(z) = A + B,  A = lam*relu(z)          (ACT Relu or DVE tensor_scalar)
                          B = min(lam*alp*e^z, lam*alp) - lam*alp
                            (ACT Exp(z + ln(lam*alp)) then DVE min/add)
    using exp(min(z,0)) == min(exp(z), 1) so no extra PSUM pass is needed.
  - The first-layer gather features[:, tf_idx] is compile-time data movement; it
    is laid out on the host into the exact [pack, row, batch] bf16 tiles the
    TensorEngine streams (this toolchain has no HIPI ucode, so the on-device
    dma_gather instruction is unavailable; the device still reads every gathered
    byte from HBM either way).
"""

import sys
import numpy as np

if "/opt/trn_rl_repo" not in sys.path:
    sys.path.insert(0, "/opt/trn_rl_repo")

import ml_dtypes

BF16 = ml_dtypes.bfloat16

G, W, K, T, B, D = 20000, 8, 16, 1500, 256, 2
NCORES = 8
GC = G // NCORES            # 2500 genes per core
GP = 2560                   # padded genes per core
NP = GP // 8                # 320 L1 packs
NG = GP // 16               # 160 groups
NU = GP // 32               # 80 units
NW = NU // 2                # 40 output windows (64 genes each)
NS = NU // 8                # 10 supers (8 units each)

LAM = 1.0507009873554805
ALPHA = 1.6732632423543772
LA = LAM * ALPHA
C0 = float(np.log(LA))      # Exp bias: exp(z + C0) = LA * e^z

_CACHE = {}
_DISABLE = set()   # debug: subsets of {'act','dve','evac'}


def _build(reps=1):
    import concourse.bass as bass
    import concourse.mybir as mybir

    f32 = mybir.dt.float32
    bf16 = mybir.dt.bfloat16
    Alu = mybir.AluOpType
    Act = mybir.ActivationFunctionType

    nc = bass.Bass()

    def reg_const(value, dtype=f32):
        t = nc.alloc_sbuf_tensor(f"const-{dtype.name}-{value}", [128, 1], dtype)
        nc.gpsimd.memset(t.ap(), value)
        nc.const_aps.aps[(dtype, value)] = t.ap()

    reg_const(C0)
    nc.all_engine_barrier()

    xg_d = nc.declare_dram_parameter("xg", [NS, 128, 32, 256], bf16, isOutput=False)
    w1_d = nc.declare_dram_parameter("w1", [NS, 128, 32, 64], bf16, isOutput=False)
    wm2_d = nc.declare_dram_parameter("wm2", [NS, 128, 16, 128], bf16, isOutput=False)
    wm3_d = nc.declare_dram_parameter("wm3", [NS, 128, 16, 128], bf16, isOutput=False)
    wf_d = nc.declare_dram_parameter("wf", [NS, 128, 16, 16], bf16, isOutput=False)
    out_d = nc.declare_dram_parameter("out", [NW, 128, 256], f32, isOutput=True)

    from contextlib import ExitStack
    with ExitStack() as ctx:
        block = ctx.enter_context(nc.Block())
        def sb(name, shape, dt=bf16):
            return ctx.enter_context(nc.sbuf_tensor(name, shape, dt))
        def ps(name):
            return ctx.enter_context(nc.psum_tensor(name, [128, 512], f32))
        def sem(name):
            return ctx.enter_context(nc.semaphore(name))
        xg_sb = sb("xg_sb", [128, 2, 32, 256]); w1_sb = sb("w1_sb", [128, 2, 32, 64])
        wm2_sb = sb("wm2_sb", [128, 2, 16, 128]); wm3_sb = sb("wm3_sb", [128, 2, 16, 128])
        wf_sb = sb("wf_sb", [128, 2, 16, 16])
        NB = _CACHE.get("nbuf", 3)   # stream-tile buffer depth
        e1_sb = sb("e1_sb", [128, NB, 512]); e2_sb = sb("e2_sb", [128, NB, 512]); e3_sb = sb("e3_sb", [128, NB, 512])
        a1_sb = sb("a1_sb", [128, NB, 512]); a2_sb = sb("a2_sb", [128, NB, 512]); a3_sb = sb("a3_sb", [128, NB, 512])
        b1_sb = sb("b1_sb", [128, NB, 512]); b2_sb = sb("b2_sb", [128, NB, 512]); b3_sb = sb("b3_sb", [128, NB, 512])
        o_sb = sb("o_sb", [128, 2, 256], f32)
        z1a = ps("z1"); z1b = ps("z1b"); z2a = ps("z2"); z2b = ps("z2b")
        z3a = ps("z3"); z3b = ps("z3b")
        zfa = ctx.enter_context(nc.psum_tensor("zfa", [128, 256], f32))
        zfb = ctx.enter_context(nc.psum_tensor("zfb", [128, 256], f32))
        zf = (zfa, zfb)
        w_sems = (sem("w_sem0"), sem("w_sem1"))
        pool_sem = sem("pool_sem")
        o_sems = (sem("o_sem0"), sem("o_sem1"))
        pe_sem = sem("pe_sem"); act_sem = sem("act_sem")
        dve_sem = sem("dve_sem"); evac_sem = sem("evac_sem")

        z1 = (z1a, z1b)
        z2 = (z2a, z2b)
        z3 = (z3a, z3b)

        def wge(eng, sem, thr):
            if thr > 0:
                eng.wait_ge(sem, thr)

        # ---- two-phase: plan op orders per engine, assign cumulative sem
        # indices, then emit.  Skewed software pipeline: PE iteration t runs
        # L1(t), L2(t-1), L3(t-2), Lf(t-3) so PE never blocks on the
        # ACT->DVE stream chain of the same unit.
        NUT = NU * reps
        SK = 1  # pipeline skew (iterations between layer stages)
        # A1 runs on ACT for some units (load balance), else DVE
        _pat = _CACHE.get("a1_pattern", ())
        a1_act = lambda u: (u % 8) in _pat
        pe_ops, act_ops, dve_ops, pool_ops = [], [], [], []
        for t in range(NUT + 3 * SK):
            if t < NUT:
                pe_ops.append(("L1", t))
                act_ops.append(("E1", t))
                if a1_act(t):
                    act_ops.append(("A1", t))
                else:
                    dve_ops.append(("A1", t))
                pool_ops.append(("B1", t))
            if 0 <= t - SK < NUT:
                pe_ops.append(("L2", t - SK))
                act_ops.append(("E2", t - SK))
                dve_ops.append(("A2", t - SK))
                pool_ops.append(("B2", t - SK))
            if 0 <= t - 2 * SK < NUT:
                pe_ops.append(("L3", t - 2 * SK))
                act_ops.append(("E3", t - 2 * SK))
                dve_ops.append(("A3", t - 2 * SK))
                pool_ops.append(("B3", t - 2 * SK))
            if 0 <= t - 3 * SK < NUT:
                pe_ops.append(("Lf", t - 3 * SK))
                if (t - 3 * SK) % 2 == 1:
                    dve_ops.append(("evac", (t - 3 * SK) // 2))
        ipe = {op: n + 1 for n, op in enumerate(pe_ops)}
        iact = {op: n + 1 for n, op in enumerate(act_ops)}
        ipool = {op: n + 1 for n, op in enumerate(pool_ops)}
        # evac ops increment evac_sem, not dve_sem -> separate numbering
        idve = {}
        ndve = 0
        for op in dve_ops:
            if op[0] != "evac":
                ndve += 1
                idve[op] = ndve

        def wge(eng, sem, thr):
            if thr > 0:
                eng.wait_ge(sem, thr)

        def wop(eng, sem, table, op):
            if table is iact and "act" in _DISABLE:
                return
            if table is idve and "dve" in _DISABLE:
                return
            if op in table:
                eng.wait_ge(sem, table[op])

        def wait_a1(eng, u):
            if ("A1", u) in iact:
                wop(eng, act_sem, iact, ("A1", u))
            else:
                wop(eng, dve_sem, idve, ("A1", u))

        @block.sync
        def _(sync):
            for s in range(NS * reps):
                # super buffers (s % 2) free once super s-2 fully consumed
                wop(sync, pe_sem, ipe, ("Lf", 8 * (s - 1) - 1))
                ws = w_sems[s % 2]
                sync.dma_start(out=xg_sb[:, s % 2], in_=xg_d[s % NS]).then_inc(ws, 16)
                sync.dma_start(out=w1_sb[:, s % 2], in_=w1_d[s % NS]).then_inc(ws, 16)
                sync.dma_start(out=wm2_sb[:, s % 2], in_=wm2_d[s % NS]).then_inc(ws, 16)
                sync.dma_start(out=wm3_sb[:, s % 2], in_=wm3_d[s % NS]).then_inc(ws, 16)
                sync.dma_start(out=wf_sb[:, s % 2], in_=wf_d[s % NS]).then_inc(ws, 16)
                if s >= 1 and "evac" not in _DISABLE and "dve" not in _DISABLE:
                    for v in range(8 * (s - 1) // 2, 8 * s // 2):
                        sync.wait_ge(evac_sem, v + 3)
                        sync.dma_start(out=out_d[v % NW], in_=o_sb[:, v % 2]).then_inc(o_sems[v % 2], 16)
            if "evac" not in _DISABLE and "dve" not in _DISABLE:
                for v in range(8 * (NS * reps - 1) // 2, NW * reps):
                    sync.wait_ge(evac_sem, v + 3)
                    sync.dma_start(out=out_d[v % NW], in_=o_sb[:, v % 2]).then_inc(o_sems[v % 2], 16)
                sync.wait_ge(o_sems[0], 16 * (NW * reps // 2))
                sync.wait_ge(o_sems[1], 16 * (NW * reps - NW * reps // 2))

        @block.tensor
        def _(tensor):
            loaded_super = -1
            for kind, u in pe_ops:
                i = u % 2
                ud = u % NU
                s = ud // 8
                j = (u // 8) % 2
                if kind == "L1":
                    sg = u // 8
                    if sg > loaded_super:
                        tensor.wait_ge(w_sems[sg % 2], 80 * (sg // 2 + 1))
                        loaded_super = sg
                    if u - 2 >= 0:
                        wait_a1(tensor, u - 2)   # implies E1(u-2) too
                    for m in range(4):
                        lp = (4 * ud + m) - 32 * s
                        mm = tensor.matmul(
                            z1[i][(m % 2) * 64:(m % 2) * 64 + 64,
                                  (m // 2) * 256:(m // 2) * 256 + 256],
                            w1_sb[:, j, lp, :],
                            xg_sb[:, j, lp, :],
                            start=True, stop=True,
                            tile_position=(0, (m % 2) * 64),
                        )
                    mm.then_inc(pe_sem, 1)
                elif kind == "L2":
                    # B-stream first: POOL's B1 lands ~300ns before DVE's A1
                    wop(tensor, pool_sem, ipool, ("B1", u))
                    wop(tensor, dve_sem, idve, ("A2", u - 2))   # implies E2(u-2)
                    ib = u % NB
                    ln0 = (2 * ud) - 16 * s
                    tensor.matmul(z2[i][:, 0:256], wm2_sb[:, j, ln0, :],
                                  b1_sb[:, ib, 0:256], start=True, stop=False)
                    wait_a1(tensor, u)
                    tensor.matmul(z2[i][:, 0:256], wm2_sb[:, j, ln0, :],
                                  a1_sb[:, ib, 0:256], start=False, stop=True)
                    tensor.matmul(z2[i][:, 256:512], wm2_sb[:, j, ln0 + 1, :],
                                  b1_sb[:, ib, 256:512], start=True, stop=False)
                    mm = tensor.matmul(z2[i][:, 256:512], wm2_sb[:, j, ln0 + 1, :],
                                       a1_sb[:, ib, 256:512], start=False, stop=True)
                    mm.then_inc(pe_sem, 1)
                elif kind == "L3":
                    wop(tensor, pool_sem, ipool, ("B2", u))
                    wop(tensor, dve_sem, idve, ("A3", u - 2))   # implies E3(u-2)
                    ib = u % NB
                    ln0 = (2 * ud) - 16 * s
                    tensor.matmul(z3[i][:, 0:256], wm3_sb[:, j, ln0, :],
                                  b2_sb[:, ib, 0:256], start=True, stop=False)
                    wop(tensor, dve_sem, idve, ("A2", u))
                    tensor.matmul(z3[i][:, 0:256], wm3_sb[:, j, ln0, :],
                                  a2_sb[:, ib, 0:256], start=False, stop=True)
                    tensor.matmul(z3[i][:, 256:512], wm3_sb[:, j, ln0 + 1, :],
                                  b2_sb[:, ib, 256:512], start=True, stop=False)
                    mm = tensor.matmul(z3[i][:, 256:512], wm3_sb[:, j, ln0 + 1, :],
                                       a2_sb[:, ib, 256:512], start=False, stop=True)
                    mm.then_inc(pe_sem, 1)
                else:  # Lf
                    v = u // 2
                    wop(tensor, pool_sem, ipool, ("B3", u))
                    if "dve" not in _DISABLE:
                        # zf[v%2] free: its memset done (+2) and evac(v-2) done
                        wge(tensor, evac_sem, max(v - 1, 0) + 2)
                    first = True
                    for g in range(2):
                        n = 2 * u + g
                        ln = (2 * ud + g) - 16 * s
                        strip = n % 4
                        sl = slice(g * 256, g * 256 + 256)
                        zout = zf[v % 2][strip * 32:strip * 32 + 16, 0:256]
                        tensor.matmul(zout, wf_sb[:, j, ln, :], b3_sb[:, u % NB, sl],
                                      start=True, stop=False, tile_position=(0, strip * 32))
                        if first:
                            wop(tensor, dve_sem, idve, ("A3", u))
                            first = False
                        mm = tensor.matmul(zout, wf_sb[:, j, ln, :], a3_sb[:, u % NB, sl],
                                           start=False, stop=True, tile_position=(0, strip * 32))
                    mm.then_inc(pe_sem, 1)

        @block.scalar
        def _(scalar):
            if "act" in _DISABLE:
                return
            for kind, u in act_ops:
                i = u % 2
                ib = u % NB
                # buffer-free (B*/A1 of u-NB) is implied: pe>=L1(u) comes after
                # PE L2/L3/Lf(u-NB), which waited on those ops (NB >= 2).
                if kind == "E1":
                    wop(scalar, pe_sem, ipe, ("L1", u))
                    scalar.activation(e1_sb[:, ib, :], z1[i][:], Act.Exp,
                                      bias=C0, scale=1.0).then_inc(act_sem, 1)
                elif kind == "A1":
                    scalar.activation(a1_sb[:, ib, :], z1[i][:], Act.Relu,
                                      bias=0.0, scale=LAM).then_inc(act_sem, 1)
                elif kind == "E2":
                    wop(scalar, pe_sem, ipe, ("L2", u))
                    scalar.activation(e2_sb[:, ib, :], z2[i][:], Act.Exp,
                                      bias=C0, scale=1.0).then_inc(act_sem, 1)
                else:  # E3
                    wop(scalar, pe_sem, ipe, ("L3", u))
                    scalar.activation(e3_sb[:, ib, :], z3[i][:], Act.Exp,
                                      bias=C0, scale=1.0).then_inc(act_sem, 1)

        @block.vector
        def _(vector):
            vector.memset(zfa[:], 0.0).then_inc(evac_sem, 1)
            vector.memset(zfb[:], 0.0).then_inc(evac_sem, 1)
            if "dve" in _DISABLE:
                return
            for kind, u in dve_ops:
                i = u % 2
                ib = u % NB
                # act-E(u) wait implies pe-L(u) (E waited on it); buffer-free
                # (pe of u-NB) implied by the same act wait transitively.
                if kind == "A1":
                    wop(vector, act_sem, iact, ("E1", u))
                    vector.tensor_scalar(a1_sb[:, ib, :], z1[i][:], 0.0, LAM,
                                         Alu.max, Alu.mult).then_inc(dve_sem, 1)
                elif kind == "A2":
                    wop(vector, act_sem, iact, ("E2", u))
                    vector.tensor_scalar(a2_sb[:, ib, :], z2[i][:], 0.0, LAM,
                                         Alu.max, Alu.mult).then_inc(dve_sem, 1)
                elif kind == "A3":
                    wop(vector, act_sem, iact, ("E3", u))
                    vector.tensor_scalar(a3_sb[:, ib, :], z3[i][:], 0.0, LAM,
                                         Alu.max, Alu.mult).then_inc(dve_sem, 1)
                else:  # evac, u = window v
                    if "evac" in _DISABLE:
                        continue
                    v = u
                    wop(vector, pe_sem, ipe, ("Lf", 2 * v + 1))
                    wge(vector, o_sems[v % 2], 16 * (v // 2))
                    vector.tensor_copy(
                        o_sb[:, v % 2], zf[v % 2][:],
                    ).then_inc(evac_sem, 1)

        @block.gpsimd
        def _(gpsimd):
            if "dve" in _DISABLE:
                return
            for kind, u in pool_ops:
                i = u % 2
                ib = u % NB
                if kind == "B1":
                    wop(gpsimd, act_sem, iact, ("E1", u))
                    gpsimd.tensor_scalar(b1_sb[:, ib, :], e1_sb[:, ib, :], LA, -LA,
                                         Alu.min, Alu.add).then_inc(pool_sem, 1)
                elif kind == "B2":
                    wop(gpsimd, act_sem, iact, ("E2", u))
                    gpsimd.tensor_scalar(b2_sb[:, ib, :], e2_sb[:, ib, :], LA, -LA,
                                         Alu.min, Alu.add).then_inc(pool_sem, 1)
                else:  # B3
                    wop(gpsimd, act_sem, iact, ("E3", u))
                    gpsimd.tensor_scalar(b3_sb[:, ib, :], e3_sb[:, ib, :], LA, -LA,
                                         Alu.min, Alu.add).then_inc(pool_sem, 1)

    return nc


def _prepare_core_inputs(features, tf_idx, W1, Wm, Wf):
    """Host-side layout: gather + block-diagonal packing, all bf16."""
    fbf = features.astype(BF16)
    maps = []
    for c in range(NCORES):
        g0 = c * GC
        tf_l = np.zeros((GP, K), np.int64)
        tf_l[:GC] = tf_idx[g0:g0 + GC]
        W1_l = np.zeros((GP, W, K), np.float32)
        W1_l[:GC] = W1[g0:g0 + GC]
        Wm_l = np.zeros((D, GP, W, W), np.float32)
        Wm_l[:, :GC] = Wm[:, g0:g0 + GC]
        Wf_l = np.zeros((GP, W), np.float32)
        Wf_l[:GC] = Wf[g0:g0 + GC]

        # xg: [NS, 128, 32, 256]  row q=16j+k of pack p = features[:, tf[8p+j, k]]
        gath = fbf[:, tf_l.reshape(-1)]                     # [B, GP*K] bf16
        xg = np.ascontiguousarray(gath.T).reshape(NP, 128, 256)
        xg = np.ascontiguousarray(
            xg.reshape(NS, 32, 128, 256).transpose(0, 2, 1, 3))

        jj = np.arange(8)
        w1b = np.zeros((NP, 8, K, 8, W), np.float32)
        w1b[:, jj, :, jj, :] = W1_l.reshape(NP, 8, W, K).transpose(
            0, 1, 3, 2).transpose(1, 0, 2, 3)
        w1b = w1b.reshape(NP, 128, 64).astype(BF16)
        w1b = np.ascontiguousarray(
            w1b.reshape(NS, 32, 128, 64).transpose(0, 2, 1, 3))

        j16 = np.arange(16)
        wmb = []
        for l in range(D):
            t = np.zeros((NG, 16, W, 16, W), np.float32)
            t[:, j16, :, j16, :] = Wm_l[l].reshape(NG, 16, W, W).transpose(
                0, 1, 3, 2).transpose(1, 0, 2, 3)
            t = t.reshape(NG, 128, 128).astype(BF16)
            wmb.append(np.ascontiguousarray(
                t.reshape(NS, 16, 128, 128).transpose(0, 2, 1, 3)))

        wfb = np.zeros((NG, 16, W, 16), np.float32)
        wfb[:, j16, :, j16] = Wf_l.reshape(NG, 16, W).transpose(1, 0, 2)
        wfb = wfb.reshape(NG, 128, 16).astype(BF16)
        wfb = np.ascontiguousarray(
            wfb.reshape(NS, 16, 128, 16).transpose(0, 2, 1, 3))

        maps.append({"xg": xg, "w1": w1b, "wm2": wmb[0], "wm3": wmb[1], "wf": wfb})
    return maps


def _assemble(results):
    """Per-core out [NW, 128, 256] -> full [B, G] f32."""
    out = np.empty((B, G), np.float32)
    for c, r in enumerate(results):
        oc = np.asarray(r["out"])                      # [NW, 128, 256]
        # window v, strip g (partitions 32g..32g+16) = genes 16*(4v+g)..+16
        genes = oc.reshape(NW, 4, 32, 256)[:, :, :16, :]   # [NW, 4, 16, 256]
        genes = genes.reshape(GP, 256)[:GC]                # [2500, B]
        out[:, c * GC:(c + 1) * GC] = genes.T
    return out


def kernel(features, tf_idx, W1, b1, Wm, bm, Wf, bf):
    from concourse.bass_utils import run_bass_kernel_spmd

    features = np.asarray(features, np.float32)
    tf_idx = np.asarray(tf_idx)
    assert not np.any(np.asarray(b1)) and not np.any(np.asarray(bm)) \
        and not np.any(np.asarray(bf)), "nonzero biases not supported"

    if "nc" not in _CACHE:
        _CACHE["nc"] = _build()
    nc = _CACHE["nc"]

    in_maps = _prepare_core_inputs(
        features, tf_idx, np.asarray(W1, np.float32),
        np.asarray(Wm, np.float32), np.asarray(Wf, np.float32))

    res = run_bass_kernel_spmd(nc, in_maps, list(range(NCORES)))
    return _assemble(res.results)



# revision 11
# speedup vs baseline: 3.6010x; 3.6010x over previous
"""Trainium2 Bass kernel for the per-gene sparse-decoder MLP.

Math (per gene g): h = selu(features[:, tf_idx[g]] @ W1[g].T); h = selu(h @ Wm[i,g].T) x2;
out[:, g] = h @ Wf[g].  Genes are independent -> shard G=20000 over 8 cores (2500 each).

Layout per core (gene dim padded 2500 -> 2560, 10 "supers" of 256 genes):
  Activations live as [node-rows on partitions, batch on free].  SELU is computed
  as two streams accumulated by the next layer's matmul pair in PSUM:
      selu(z) = A + B,  A = lam*relu(z)                      (ACT Relu)
                        B = min(lam*alp*e^z, lam*alp) - lam*alp
                          = (E min lam*alp) + (-lam*alp)     (DVE tensor_scalar)
      with E = lam*alp*e^z = Exp(z + ln(lam*alp))            (ACT Exp)
  GPSIMD is deliberately unused: HW-measured tensor_scalar there is ~7.5us/op
  (16x the cost model), and it shares an SBUF port with DVE.

Schedule: LAYER-MAJOR within each super -- PE does all 8 L1 banks, then all 8
L2 banks, all 8 L3 banks, then 2 Lf banks, rotating the 8 PSUM banks as one
pool.  Every cross-engine semaphore wait is satisfied ~a full phase in
advance, so engines stream without round-trip stalls (the previous
unit-interleaved schedule serialized on cross-engine wake-ups).

The first-layer gather features[:, tf_idx] is compile-time data movement; it
is laid out on the host into the exact [pack, row, batch] bf16 tiles the
TensorEngine streams.
"""

import sys
import numpy as np

if "/opt/trn_rl_repo" not in sys.path:
    sys.path.insert(0, "/opt/trn_rl_repo")

import ml_dtypes

BF16 = ml_dtypes.bfloat16

G, W, K, T, B, D = 20000, 8, 16, 1500, 256, 2
NCORES = 8
GC = G // NCORES            # 2500 genes per core
GP = 2560                   # padded genes per core
NP = GP // 8                # 320 L1 packs
NG = GP // 16               # 160 16-gene groups
NU = GP // 32               # 80 units (32 genes)
NW = NU // 2                # 40 output windows (64 genes each)
NS = NU // 8                # 10 supers (8 units each)
NC = 2 * NS                 # 20 output chunks ([128, 512] = 2 windows)

LAM = 1.0507009873554805
ALPHA = 1.6732632423543772
LA = LAM * ALPHA
C0 = float(np.log(LA))      # Exp bias: exp(z + C0) = LA * e^z

XB = 3                      # xg stream buffer depth (supers)
ED = 8                      # e-tile buffer depth (banks)

_CACHE = {}


def _build(reps=1):
    import concourse.bass as bass
    import concourse.mybir as mybir

    f32 = mybir.dt.float32
    bf16 = mybir.dt.bfloat16
    Alu = mybir.AluOpType
    Act = mybir.ActivationFunctionType

    nc = bass.Bass()

    def reg_const(value, dtype=f32):
        t = nc.alloc_sbuf_tensor(f"const-{dtype.name}-{value}", [128, 1], dtype)
        nc.gpsimd.memset(t.ap(), value)
        nc.const_aps.aps[(dtype, value)] = t.ap()

    reg_const(C0)
    nc.all_engine_barrier()

    xg_d = nc.declare_dram_parameter("xg", [NS, 128, 32, 256], bf16, isOutput=False)
    w1_d = nc.declare_dram_parameter("w1", [NS, 128, 32, 64], bf16, isOutput=False)
    wm2_d = nc.declare_dram_parameter("wm2", [NS, 128, 16, 128], bf16, isOutput=False)
    wm3_d = nc.declare_dram_parameter("wm3", [NS, 128, 16, 128], bf16, isOutput=False)
    wf_d = nc.declare_dram_parameter("wf", [NS, 128, 16, 16], bf16, isOutput=False)
    out_d = nc.declare_dram_parameter("out", [NC, 128, 512], f32, isOutput=True)

    TS = NS * reps           # total supers

    # ---------------- global schedule plan ----------------
    # PE fill sequence: per super: 8xL1, 8xL2, 8xL3, 2xLf  (26 fills/super)
    # Each fill targets PSUM bank (fill_idx % 8).
    fills = []               # (kind, s, u_or_chunk)
    for ss in range(TS):
        for u in range(8):
            fills.append(("L1", ss, u))
        for u in range(8):
            fills.append(("L2", ss, u))
        for u in range(8):
            fills.append(("L3", ss, u))
        for c in range(2):
            fills.append(("Lf", ss, c))
    ipe = {f: n + 1 for n, f in enumerate(fills)}      # pe_sem after fill f

    # ACT op order: for each z-fill (L1/L2/L3) in fill order: E then A.
    act_ops = []
    for f in fills:
        if f[0] != "Lf":
            act_ops.append(("E", f))
            act_ops.append(("A", f))
    iact = {op: n + 1 for n, op in enumerate(act_ops)}

    # DVE op order: B per z-fill, evac per Lf fill, in fill order.
    dve_ops = []
    for f in fills:
        if f[0] == "Lf":
            dve_ops.append(("V", f))
        else:
            dve_ops.append(("B", f))
    idve = {op: n + 1 for n, op in enumerate(dve_ops)}

    # z-fill index (for e-slot rotation) and per-fill bank
    zfills = [f for f in fills if f[0] != "Lf"]
    izf = {f: n for n, f in enumerate(zfills)}
    bank_of = {f: n % 8 for n, f in enumerate(fills)}
    # chunk index for Lf fills / output
    lffills = [f for f in fills if f[0] == "Lf"]
    ichunk = {f: n for n, f in enumerate(lffills)}

    # DMA plan: per super 5 input DMAs (xg,w1,wm2,wm3,wf) -> w_sem += 80
    # output: per super 2 chunks -> o_sem += 16 each
    from contextlib import ExitStack
    with ExitStack() as ctx:
        block = ctx.enter_context(nc.Block())

        def sb(name, shape, dt=bf16):
            return ctx.enter_context(nc.sbuf_tensor(name, shape, dt))

        xg_sb = sb("xg_sb", [128, XB, 32, 256])
        w1_sb = sb("w1_sb", [128, 2, 32, 64])
        wm2_sb = sb("wm2_sb", [128, 2, 16, 128])
        wm3_sb = sb("wm3_sb", [128, 2, 16, 128])
        wf_sb = sb("wf_sb", [128, 2, 16, 16])
        e_sb = sb("e_sb", [128, ED, 512])
        a_sb = [sb(f"a{l}_sb", [128, 8, 512]) for l in (1, 2, 3)]
        b_sb = [sb(f"b{l}_sb", [128, 8, 512]) for l in (1, 2, 3)]
        o_sb = sb("o_sb", [128, 6, 512], f32)
        banks = [ctx.enter_context(nc.psum_tensor(f"zb{i}", [128, 512], f32))
                 for i in range(8)]

        pe_sem = ctx.enter_context(nc.semaphore("pe_sem"))
        act_sem = ctx.enter_context(nc.semaphore("act_sem"))
        dve_sem = ctx.enter_context(nc.semaphore("dve_sem"))
        # parity-alternating DMA sems: a sem may only take increments from one
        # DMA batch at a time across a waited threshold (sim race detector)
        w_sems = (ctx.enter_context(nc.semaphore("w_sem0")),
                  ctx.enter_context(nc.semaphore("w_sem1")))
        o_sems = (ctx.enter_context(nc.semaphore("o_sem0")),
                  ctx.enter_context(nc.semaphore("o_sem1")))

        def ab(layer):
            return a_sb[layer - 1], b_sb[layer - 1]

        @block.sync
        def _(sync):
            for ss in range(TS + 2):
                if ss < TS:
                    s = ss % NS
                    ws = w_sems[ss % 2]
                    # previous same-parity super's inputs fully landed
                    if ss >= 2:
                        sync.wait_ge(ws, 80 * (ss // 2))
                    # xg slot ss%XB free once PE finished L1 of super ss-XB
                    if ss >= XB:
                        sync.wait_ge(pe_sem, ipe[("L1", ss - XB, 7)])
                    sync.dma_start(out=xg_sb[:, ss % XB], in_=xg_d[s]).then_inc(ws, 16)
                    # weight slots ss%2 free once super ss-2 consumed them
                    if ss >= 2:
                        sync.wait_ge(pe_sem, ipe[("Lf", ss - 2, 1)])
                    sync.dma_start(out=w1_sb[:, ss % 2], in_=w1_d[s]).then_inc(ws, 16)
                    sync.dma_start(out=wm2_sb[:, ss % 2], in_=wm2_d[s]).then_inc(ws, 16)
                    sync.dma_start(out=wm3_sb[:, ss % 2], in_=wm3_d[s]).then_inc(ws, 16)
                    sync.dma_start(out=wf_sb[:, ss % 2], in_=wf_d[s]).then_inc(ws, 16)
                if ss >= 2:
                    # outputs of super ss-2
                    for c in range(2):
                        q = 2 * (ss - 2) + c
                        os_ = o_sems[q % 2]
                        if q >= 2:
                            sync.wait_ge(os_, 16 * (q // 2))
                        sync.wait_ge(dve_sem, idve[("V", ("Lf", ss - 2, c))])
                        sync.dma_start(out=out_d[q % NC], in_=o_sb[:, q % 6]).then_inc(os_, 16)
            sync.wait_ge(o_sems[0], 16 * TS)
            sync.wait_ge(o_sems[1], 16 * TS)

        @block.tensor
        def _(tensor):
            for fi, f in enumerate(fills):
                kind, ss, x = f
                s = ss % NS
                j = ss % 2
                bank = banks[bank_of[f]]
                # bank free: drain of fill fi-8 complete
                if fi >= 8:
                    prev = fills[fi - 8]
                    if prev[0] == "Lf":
                        tensor.wait_ge(dve_sem, idve[("V", prev)])
                    else:
                        tensor.wait_ge(act_sem, iact[("A", prev)])
                if kind == "L1":
                    if x == 0:
                        # all 5 input DMAs of super ss landed (per-DMA count
                        # thresholds are racy: SDMA engines interleave incs)
                        tensor.wait_ge(w_sems[ss % 2], 80 * (ss // 2 + 1))
                    for m in range(4):
                        lp = 4 * x + m
                        mm = tensor.matmul(
                            bank[(m % 2) * 64:(m % 2) * 64 + 64,
                                 (m // 2) * 256:(m // 2) * 256 + 256],
                            w1_sb[:, j, lp, :],
                            xg_sb[:, ss % XB, lp, :],
                            start=True, stop=True,
                            tile_position=(0, (m % 2) * 64),
                        )
                    mm.then_inc(pe_sem, 1)
                elif kind in ("L2", "L3"):
                    layer = 2 if kind == "L2" else 3
                    wsb = wm2_sb if kind == "L2" else wm3_sb
                    asrc, bsrc = ab(layer - 1)
                    if x == 0:
                        # h tiles of this layer ready: A/B of last feeding fill done
                        feed = (("L1" if kind == "L2" else "L2"), ss, 7)
                        tensor.wait_ge(act_sem, iact[("A", feed)])
                        tensor.wait_ge(dve_sem, idve[("B", feed)])
                    for gg in range(2):
                        ln = 2 * x + gg
                        sl = slice(gg * 256, gg * 256 + 256)
                        tensor.matmul(bank[:, sl], wsb[:, j, ln, :],
                                      bsrc[:, x, sl], start=True, stop=False)
                        mm = tensor.matmul(bank[:, sl], wsb[:, j, ln, :],
                                           asrc[:, x, sl], start=False, stop=True)
                    mm.then_inc(pe_sem, 1)
                else:  # Lf chunk x (2 windows)
                    asrc, bsrc = ab(3)
                    if x == 0:
                        feed = ("L3", ss, 7)
                        tensor.wait_ge(act_sem, iact[("A", feed)])
                        tensor.wait_ge(dve_sem, idve[("B", feed)])
                    for h in range(2):          # window within chunk
                        v = 2 * x + h           # window within super
                        for n in range(2):      # unit within window
                            u = 2 * v + n
                            for gg in range(2):
                                strip = 2 * n + gg
                                ln = 2 * u + gg
                                sl = slice(gg * 256, gg * 256 + 256)
                                zout = bank[strip * 32:strip * 32 + 16,
                                            h * 256:h * 256 + 256]
                                tensor.matmul(zout, wf_sb[:, j, ln, :],
                                              bsrc[:, u, sl], start=True, stop=False,
                                              tile_position=(0, strip * 32))
                                mm = tensor.matmul(zout, wf_sb[:, j, ln, :],
                                                   asrc[:, u, sl], start=False, stop=True,
                                                   tile_position=(0, strip * 32))
                    mm.then_inc(pe_sem, 1)

        @block.scalar
        def _(scalar):
            for op, f in act_ops:
                kind, ss, u = f
                layer = {"L1": 1, "L2": 2, "L3": 3}[kind]
                asrc, bsrc = ab(layer)
                bank = banks[bank_of[f]]
                k = izf[f]
                if op == "E":
                    scalar.wait_ge(pe_sem, ipe[f])
                    if k >= ED:
                        # e-slot free once B of fill k-ED consumed it
                        scalar.wait_ge(dve_sem, idve[("B", zfills[k - ED])])
                    scalar.activation(e_sb[:, k % ED, :], bank[:], Act.Exp,
                                      bias=C0, scale=1.0).then_inc(act_sem, 1)
                else:
                    scalar.activation(asrc[:, u, :], bank[:], Act.Relu,
                                      bias=0.0, scale=LAM).then_inc(act_sem, 1)

        @block.vector
        def _(vector):
            for op, f in dve_ops:
                kind, ss, x = f
                if op == "B":
                    layer = {"L1": 1, "L2": 2, "L3": 3}[kind]
                    asrc, bsrc = ab(layer)
                    k = izf[f]
                    vector.wait_ge(act_sem, iact[("E", f)])
                    vector.tensor_scalar(bsrc[:, x, :], e_sb[:, k % ED, :], LA, -LA,
                                         Alu.min, Alu.add).then_inc(dve_sem, 1)
                else:  # evac
                    q = ichunk[f]
                    vector.wait_ge(pe_sem, ipe[f])
                    if q >= 6:
                        # o_sb slot free once out-DMA q-6 (same parity) done
                        vector.wait_ge(o_sems[q % 2], 16 * ((q - 6) // 2 + 1))
                    vector.tensor_copy(o_sb[:, q % 6], banks[bank_of[f]][:],
                                       ).then_inc(dve_sem, 1)

    return nc


def _prepare_core_inputs(features, tf_idx, W1, Wm, Wf):
    """Host-side layout: gather + block-diagonal packing, all bf16."""
    fbf = features.astype(BF16)
    maps = []
    for c in range(NCORES):
        g0 = c * GC
        tf_l = np.zeros((GP, K), np.int64)
        tf_l[:GC] = tf_idx[g0:g0 + GC]
        W1_l = np.zeros((GP, W, K), np.float32)
        W1_l[:GC] = W1[g0:g0 + GC]
        Wm_l = np.zeros((D, GP, W, W), np.float32)
        Wm_l[:, :GC] = Wm[:, g0:g0 + GC]
        Wf_l = np.zeros((GP, W), np.float32)
        Wf_l[:GC] = Wf[g0:g0 + GC]

        # xg: [NS, 128, 32, 256]  row q=16j+k of pack p = features[:, tf[8p+j, k]]
        gath = fbf[:, tf_l.reshape(-1)]                     # [B, GP*K] bf16
        xg = np.ascontiguousarray(gath.T).reshape(NP, 128, 256)
        xg = np.ascontiguousarray(
            xg.reshape(NS, 32, 128, 256).transpose(0, 2, 1, 3))

        jj = np.arange(8)
        w1b = np.zeros((NP, 8, K, 8, W), np.float32)
        w1b[:, jj, :, jj, :] = W1_l.reshape(NP, 8, W, K).transpose(
            0, 1, 3, 2).transpose(1, 0, 2, 3)
        w1b = w1b.reshape(NP, 128, 64).astype(BF16)
        w1b = np.ascontiguousarray(
            w1b.reshape(NS, 32, 128, 64).transpose(0, 2, 1, 3))

        j16 = np.arange(16)
        wmb = []
        for l in range(D):
            t = np.zeros((NG, 16, W, 16, W), np.float32)
            t[:, j16, :, j16, :] = Wm_l[l].reshape(NG, 16, W, W).transpose(
                0, 1, 3, 2).transpose(1, 0, 2, 3)
            t = t.reshape(NG, 128, 128).astype(BF16)
            wmb.append(np.ascontiguousarray(
                t.reshape(NS, 16, 128, 128).transpose(0, 2, 1, 3)))

        wfb = np.zeros((NG, 16, W, 16), np.float32)
        wfb[:, j16, :, j16] = Wf_l.reshape(NG, 16, W).transpose(1, 0, 2)
        wfb = wfb.reshape(NG, 128, 16).astype(BF16)
        wfb = np.ascontiguousarray(
            wfb.reshape(NS, 16, 128, 16).transpose(0, 2, 1, 3))

        maps.append({"xg": xg, "w1": w1b, "wm2": wmb[0], "wm3": wmb[1], "wf": wfb})
    return maps


def _assemble(results):
    """Per-core out [NC, 128, 512] -> full [B, G] f32.

    Chunk q holds windows v=2q (cols 0:256) and v=2q+1 (cols 256:512);
    window strip layout: partitions 32*strip..+16 = genes 64v+16*strip..+16.
    """
    out = np.empty((B, G), np.float32)
    for c, r in enumerate(results):
        oc = np.asarray(r["out"])                          # [NC, 128, 512]
        oc = oc.reshape(NC, 4, 32, 2, 256)[:, :, :16, :, :]  # [q, strip, j, h, b]
        genes = oc.transpose(0, 3, 1, 2, 4).reshape(GP, 256)[:GC]
        out[:, c * GC:(c + 1) * GC] = genes.T
    return out


def kernel(features, tf_idx, W1, b1, Wm, bm, Wf, bf):
    from concourse.bass_utils import run_bass_kernel_spmd

    features = np.asarray(features, np.float32)
    tf_idx = np.asarray(tf_idx)
    assert not np.any(np.asarray(b1)) and not np.any(np.asarray(bm)) \
        and not np.any(np.asarray(bf)), "nonzero biases not supported"

    if "nc" not in _CACHE:
        _CACHE["nc"] = _build()
    nc = _CACHE["nc"]

    in_maps = _prepare_core_inputs(
        features, tf_idx, np.asarray(W1, np.float32),
        np.asarray(Wm, np.float32), np.asarray(Wf, np.float32))

    res = run_bass_kernel_spmd(nc, in_maps, list(range(NCORES)))
    return _assemble(res.results)


# revision 13
# speedup vs baseline: 37.8071x; 10.4991x over previous
"""Trainium2 Bass kernel for the per-gene sparse-decoder MLP.

Math (per gene g): h = selu(features[:, tf_idx[g]] @ W1[g].T); h = selu(h @ Wm[i,g].T) x2;
out[:, g] = h @ Wf[g].  Genes are independent -> shard G=20000 over 8 cores (2500 each).

Layout per core (gene dim padded 2500 -> 2560, 10 "supers" of 256 genes):
  Activations live as [node-rows on partitions, batch on free].  SELU is computed
  as two streams accumulated by the next layer's matmul pair in PSUM:
      selu(z) = A + B,  A = lam*relu(z)                      (ACT Relu)
                        B = min(lam*alp*e^z, lam*alp) - lam*alp
                          = (E min lam*alp) + (-lam*alp)     (DVE tensor_scalar)
      with E = lam*alp*e^z = Exp(z + ln(lam*alp))            (ACT Exp)
  GPSIMD is deliberately unused: HW-measured tensor_scalar there is ~7.5us/op
  (16x the cost model), and it shares an SBUF port with DVE.

Schedule: LAYER-MAJOR within each super -- PE does all 8 L1 banks, then all 8
L2 banks, all 8 L3 banks, then 2 Lf banks, rotating the 8 PSUM banks as one
pool.  Every cross-engine semaphore wait is satisfied ~a full phase in
advance, so engines stream without round-trip stalls (the previous
unit-interleaved schedule serialized on cross-engine wake-ups).

The first-layer gather features[:, tf_idx] is compile-time data movement; it
is laid out on the host into the exact [pack, row, batch] bf16 tiles the
TensorEngine streams.
"""

import sys
import numpy as np

if "/opt/trn_rl_repo" not in sys.path:
    sys.path.insert(0, "/opt/trn_rl_repo")

import ml_dtypes

BF16 = ml_dtypes.bfloat16

G, W, K, T, B, D = 20000, 8, 16, 1500, 256, 2
NCORES = 8
GC = G // NCORES            # 2500 genes per core
GP = 2560                   # padded genes per core
NP = GP // 8                # 320 L1 packs
NG = GP // 16               # 160 16-gene groups
NU = GP // 32               # 80 units (32 genes)
NW = NU // 2                # 40 output windows (64 genes each)
NS = NU // 8                # 10 supers (8 units each)
NC = 2 * NS                 # 20 output chunks ([128, 512] = 2 windows)

LAM = 1.0507009873554805
ALPHA = 1.6732632423543772
LA = LAM * ALPHA
C0 = float(np.log(LA))      # Exp bias: exp(z + C0) = LA * e^z

XB = 3                      # xg stream buffer depth (supers)
ED = 8                      # e-tile buffer depth (banks)

_CACHE = {}


def _build(reps=1):
    import concourse.bass as bass
    import concourse.mybir as mybir

    f32 = mybir.dt.float32
    bf16 = mybir.dt.bfloat16
    Alu = mybir.AluOpType
    Act = mybir.ActivationFunctionType

    nc = bass.Bass()

    def reg_const(value, dtype=f32):
        t = nc.alloc_sbuf_tensor(f"const-{dtype.name}-{value}", [128, 1], dtype)
        nc.gpsimd.memset(t.ap(), value)
        nc.const_aps.aps[(dtype, value)] = t.ap()

    reg_const(C0)
    nc.all_engine_barrier()

    xg_d = nc.declare_dram_parameter("xg", [NS, 128, 32, 256], bf16, isOutput=False)
    w1_d = nc.declare_dram_parameter("w1", [NS, 128, 32, 64], bf16, isOutput=False)
    wm2_d = nc.declare_dram_parameter("wm2", [NS, 128, 16, 128], bf16, isOutput=False)
    wm3_d = nc.declare_dram_parameter("wm3", [NS, 128, 16, 128], bf16, isOutput=False)
    wf_d = nc.declare_dram_parameter("wf", [NS, 128, 16, 16], bf16, isOutput=False)
    out_d = nc.declare_dram_parameter("out", [NC, 128, 512], f32, isOutput=True)

    TS = NS * reps           # total supers

    # ---------------- global schedule plan ----------------
    # PE fill sequence: per super: 8xL1, 8xL2, 8xL3, 2xLf  (26 fills/super)
    # Each fill targets PSUM bank (fill_idx % 8).
    fills = []               # (kind, s, u_or_chunk)
    for ss in range(TS):
        for u in range(8):
            fills.append(("L1", ss, u))
        for u in range(8):
            fills.append(("L2", ss, u))
        for u in range(8):
            fills.append(("L3", ss, u))
        for c in range(2):
            fills.append(("Lf", ss, c))
    ipe = {f: n + 1 for n, f in enumerate(fills)}      # pe_sem after fill f

    # ACT op order: for each z-fill (L1/L2/L3) in fill order: E then A.
    act_ops = []
    for f in fills:
        if f[0] != "Lf":
            act_ops.append(("E", f))
            act_ops.append(("A", f))
    iact = {op: n + 1 for n, op in enumerate(act_ops)}

    # DVE op order: B per z-fill, evac per Lf fill, in fill order.
    dve_ops = []
    for f in fills:
        if f[0] == "Lf":
            dve_ops.append(("V", f))
        else:
            dve_ops.append(("B", f))
    idve = {op: n + 1 for n, op in enumerate(dve_ops)}

    # z-fill index (for e-slot rotation) and per-fill bank
    zfills = [f for f in fills if f[0] != "Lf"]
    izf = {f: n for n, f in enumerate(zfills)}
    bank_of = {f: n % 8 for n, f in enumerate(fills)}
    # chunk index for Lf fills / output
    lffills = [f for f in fills if f[0] == "Lf"]
    ichunk = {f: n for n, f in enumerate(lffills)}

    # DMA plan: per super 5 input DMAs (xg,w1,wm2,wm3,wf) -> w_sem += 80
    # output: per super 2 chunks -> o_sem += 16 each
    from contextlib import ExitStack
    with ExitStack() as ctx:
        block = ctx.enter_context(nc.Block())

        def sb(name, shape, dt=bf16):
            return ctx.enter_context(nc.sbuf_tensor(name, shape, dt))

        xg_sb = sb("xg_sb", [128, XB, 32, 256])
        w1_sb = sb("w1_sb", [128, 2, 32, 64])
        wm2_sb = sb("wm2_sb", [128, 2, 16, 128])
        wm3_sb = sb("wm3_sb", [128, 2, 16, 128])
        wf_sb = sb("wf_sb", [128, 2, 16, 16])
        e_sb = sb("e_sb", [128, ED, 512])
        a_sb = [sb(f"a{l}_sb", [128, 8, 512]) for l in (1, 2, 3)]
        b_sb = [sb(f"b{l}_sb", [128, 8, 512]) for l in (1, 2, 3)]
        o_sb = sb("o_sb", [128, 6, 512], f32)
        banks = [ctx.enter_context(nc.psum_tensor(f"zb{i}", [128, 512], f32))
                 for i in range(8)]

        pe_sem = ctx.enter_context(nc.semaphore("pe_sem"))
        act_sem = ctx.enter_context(nc.semaphore("act_sem"))
        dve_sem = ctx.enter_context(nc.semaphore("dve_sem"))
        # parity-alternating DMA sems: a sem may only take increments from one
        # DMA batch at a time across a waited threshold (sim race detector)
        w_sems = (ctx.enter_context(nc.semaphore("w_sem0")),
                  ctx.enter_context(nc.semaphore("w_sem1")))
        o_sems = (ctx.enter_context(nc.semaphore("o_sem0")),
                  ctx.enter_context(nc.semaphore("o_sem1")))

        def ab(layer):
            return a_sb[layer - 1], b_sb[layer - 1]

        @block.sync
        def _(sync):
            for ss in range(TS + 2):
                if ss < TS:
                    s = ss % NS
                    ws = w_sems[ss % 2]
                    # previous same-parity super's inputs fully landed
                    if ss >= 2:
                        sync.wait_ge(ws, 80 * (ss // 2))
                    # xg slot ss%XB free once PE finished L1 of super ss-XB
                    if ss >= XB:
                        sync.wait_ge(pe_sem, ipe[("L1", ss - XB, 7)])
                    sync.dma_start(out=xg_sb[:, ss % XB], in_=xg_d[s]).then_inc(ws, 16)
                    # weight slots ss%2 free once super ss-2 consumed them
                    if ss >= 2:
                        sync.wait_ge(pe_sem, ipe[("Lf", ss - 2, 1)])
                    sync.dma_start(out=w1_sb[:, ss % 2], in_=w1_d[s]).then_inc(ws, 16)
                    sync.dma_start(out=wm2_sb[:, ss % 2], in_=wm2_d[s]).then_inc(ws, 16)
                    sync.dma_start(out=wm3_sb[:, ss % 2], in_=wm3_d[s]).then_inc(ws, 16)
                    sync.dma_start(out=wf_sb[:, ss % 2], in_=wf_d[s]).then_inc(ws, 16)
                if ss >= 2:
                    # outputs of super ss-2
                    for c in range(2):
                        q = 2 * (ss - 2) + c
                        os_ = o_sems[q % 2]
                        if q >= 2:
                            sync.wait_ge(os_, 16 * (q // 2))
                        sync.wait_ge(dve_sem, idve[("V", ("Lf", ss - 2, c))])
                        sync.dma_start(out=out_d[q % NC], in_=o_sb[:, q % 6]).then_inc(os_, 16)
            sync.wait_ge(o_sems[0], 16 * TS)
            sync.wait_ge(o_sems[1], 16 * TS)

        @block.tensor
        def _(tensor):
            for fi, f in enumerate(fills):
                kind, ss, x = f
                s = ss % NS
                j = ss % 2
                bank = banks[bank_of[f]]
                # bank free: drain of fill fi-8 complete
                if fi >= 8:
                    prev = fills[fi - 8]
                    if prev[0] == "Lf":
                        tensor.wait_ge(dve_sem, idve[("V", prev)])
                    else:
                        tensor.wait_ge(act_sem, iact[("A", prev)])
                if kind == "L1":
                    if x == 0:
                        # all 5 input DMAs of super ss landed (per-DMA count
                        # thresholds are racy: SDMA engines interleave incs)
                        tensor.wait_ge(w_sems[ss % 2], 80 * (ss // 2 + 1))
                    for m in range(4):
                        lp = 4 * x + m
                        mm = tensor.matmul(
                            bank[(m % 2) * 64:(m % 2) * 64 + 64,
                                 (m // 2) * 256:(m // 2) * 256 + 256],
                            w1_sb[:, j, lp, :],
                            xg_sb[:, ss % XB, lp, :],
                            start=True, stop=True,
                            tile_position=(0, (m % 2) * 64),
                        )
                    mm.then_inc(pe_sem, 1)
                elif kind in ("L2", "L3"):
                    layer = 2 if kind == "L2" else 3
                    wsb = wm2_sb if kind == "L2" else wm3_sb
                    asrc, bsrc = ab(layer - 1)
                    # h tile x of the previous layer ready (per-bank, so the
                    # next phase starts as soon as its first input is drained)
                    feed = (("L1" if kind == "L2" else "L2"), ss, x)
                    tensor.wait_ge(act_sem, iact[("A", feed)])
                    tensor.wait_ge(dve_sem, idve[("B", feed)])
                    for gg in range(2):
                        ln = 2 * x + gg
                        sl = slice(gg * 256, gg * 256 + 256)
                        tensor.matmul(bank[:, sl], wsb[:, j, ln, :],
                                      bsrc[:, x, sl], start=True, stop=False)
                        mm = tensor.matmul(bank[:, sl], wsb[:, j, ln, :],
                                           asrc[:, x, sl], start=False, stop=True)
                    mm.then_inc(pe_sem, 1)
                else:  # Lf chunk x (2 windows, units 4x..4x+3)
                    asrc, bsrc = ab(3)
                    feed = ("L3", ss, 4 * x + 3)
                    tensor.wait_ge(act_sem, iact[("A", feed)])
                    tensor.wait_ge(dve_sem, idve[("B", feed)])
                    for h in range(2):          # window within chunk
                        v = 2 * x + h           # window within super
                        for n in range(2):      # unit within window
                            u = 2 * v + n
                            for gg in range(2):
                                strip = 2 * n + gg
                                ln = 2 * u + gg
                                sl = slice(gg * 256, gg * 256 + 256)
                                zout = bank[strip * 32:strip * 32 + 16,
                                            h * 256:h * 256 + 256]
                                tensor.matmul(zout, wf_sb[:, j, ln, :],
                                              bsrc[:, u, sl], start=True, stop=False,
                                              tile_position=(0, strip * 32))
                                mm = tensor.matmul(zout, wf_sb[:, j, ln, :],
                                                   asrc[:, u, sl], start=False, stop=True,
                                                   tile_position=(0, strip * 32))
                    mm.then_inc(pe_sem, 1)

        @block.scalar
        def _(scalar):
            for op, f in act_ops:
                kind, ss, u = f
                layer = {"L1": 1, "L2": 2, "L3": 3}[kind]
                asrc, bsrc = ab(layer)
                bank = banks[bank_of[f]]
                k = izf[f]
                if op == "E":
                    scalar.wait_ge(pe_sem, ipe[f])
                    if k >= ED:
                        # e-slot free once B of fill k-ED consumed it
                        scalar.wait_ge(dve_sem, idve[("B", zfills[k - ED])])
                    scalar.activation(e_sb[:, k % ED, :], bank[:], Act.Exp,
                                      bias=C0, scale=1.0).then_inc(act_sem, 1)
                else:
                    scalar.activation(asrc[:, u, :], bank[:], Act.Relu,
                                      bias=0.0, scale=LAM).then_inc(act_sem, 1)

        @block.vector
        def _(vector):
            for op, f in dve_ops:
                kind, ss, x = f
                if op == "B":
                    layer = {"L1": 1, "L2": 2, "L3": 3}[kind]
                    asrc, bsrc = ab(layer)
                    k = izf[f]
                    vector.wait_ge(act_sem, iact[("E", f)])
                    vector.tensor_scalar(bsrc[:, x, :], e_sb[:, k % ED, :], LA, -LA,
                                         Alu.min, Alu.add).then_inc(dve_sem, 1)
                else:  # evac
                    q = ichunk[f]
                    vector.wait_ge(pe_sem, ipe[f])
                    if q >= 6:
                        # o_sb slot free once out-DMA q-6 (same parity) done
                        vector.wait_ge(o_sems[q % 2], 16 * ((q - 6) // 2 + 1))
                    vector.tensor_copy(o_sb[:, q % 6], banks[bank_of[f]][:],
                                       ).then_inc(dve_sem, 1)

    return nc


def _prepare_core_inputs(features, tf_idx, W1, Wm, Wf):
    """Host-side layout: gather + block-diagonal packing, all bf16."""
    fbf = features.astype(BF16)
    maps = []
    for c in range(NCORES):
        g0 = c * GC
        tf_l = np.zeros((GP, K), np.int64)
        tf_l[:GC] = tf_idx[g0:g0 + GC]
        W1_l = np.zeros((GP, W, K), np.float32)
        W1_l[:GC] = W1[g0:g0 + GC]
        Wm_l = np.zeros((D, GP, W, W), np.float32)
        Wm_l[:, :GC] = Wm[:, g0:g0 + GC]
        Wf_l = np.zeros((GP, W), np.float32)
        Wf_l[:GC] = Wf[g0:g0 + GC]

        # xg: [NS, 128, 32, 256]  row q=16j+k of pack p = features[:, tf[8p+j, k]]
        gath = fbf[:, tf_l.reshape(-1)]                     # [B, GP*K] bf16
        xg = np.ascontiguousarray(gath.T).reshape(NP, 128, 256)
        xg = np.ascontiguousarray(
            xg.reshape(NS, 32, 128, 256).transpose(0, 2, 1, 3))

        jj = np.arange(8)
        w1b = np.zeros((NP, 8, K, 8, W), np.float32)
        w1b[:, jj, :, jj, :] = W1_l.reshape(NP, 8, W, K).transpose(
            0, 1, 3, 2).transpose(1, 0, 2, 3)
        w1b = w1b.reshape(NP, 128, 64).astype(BF16)
        w1b = np.ascontiguousarray(
            w1b.reshape(NS, 32, 128, 64).transpose(0, 2, 1, 3))

        j16 = np.arange(16)
        wmb = []
        for l in range(D):
            t = np.zeros((NG, 16, W, 16, W), np.float32)
            t[:, j16, :, j16, :] = Wm_l[l].reshape(NG, 16, W, W).transpose(
                0, 1, 3, 2).transpose(1, 0, 2, 3)
            t = t.reshape(NG, 128, 128).astype(BF16)
            wmb.append(np.ascontiguousarray(
                t.reshape(NS, 16, 128, 128).transpose(0, 2, 1, 3)))

        wfb = np.zeros((NG, 16, W, 16), np.float32)
        wfb[:, j16, :, j16] = Wf_l.reshape(NG, 16, W).transpose(1, 0, 2)
        wfb = wfb.reshape(NG, 128, 16).astype(BF16)
        wfb = np.ascontiguousarray(
            wfb.reshape(NS, 16, 128, 16).transpose(0, 2, 1, 3))

        maps.append({"xg": xg, "w1": w1b, "wm2": wmb[0], "wm3": wmb[1], "wf": wfb})
    return maps


def _assemble(results):
    """Per-core out [NC, 128, 512] -> full [B, G] f32.

    Chunk q holds windows v=2q (cols 0:256) and v=2q+1 (cols 256:512);
    window strip layout: partitions 32*strip..+16 = genes 64v+16*strip..+16.
    """
    out = np.empty((B, G), np.float32)
    for c, r in enumerate(results):
        oc = np.asarray(r["out"])                          # [NC, 128, 512]
        oc = oc.reshape(NC, 4, 32, 2, 256)[:, :, :16, :, :]  # [q, strip, j, h, b]
        genes = oc.transpose(0, 3, 1, 2, 4).reshape(GP, 256)[:GC]
        out[:, c * GC:(c + 1) * GC] = genes.T
    return out


def kernel(features, tf_idx, W1, b1, Wm, bm, Wf, bf):
    from concourse.bass_utils import run_bass_kernel_spmd

    features = np.asarray(features, np.float32)
    tf_idx = np.asarray(tf_idx)
    assert not np.any(np.asarray(b1)) and not np.any(np.asarray(bm)) \
        and not np.any(np.asarray(bf)), "nonzero biases not supported"

    if "nc" not in _CACHE:
        _CACHE["nc"] = _build()
    nc = _CACHE["nc"]

    in_maps = _prepare_core_inputs(
        features, tf_idx, np.asarray(W1, np.float32),
        np.asarray(Wm, np.float32), np.asarray(Wf, np.float32))

    res = run_bass_kernel_spmd(nc, in_maps, list(range(NCORES)))
    return _assemble(res.results)
